# revision 23
# baseline (speedup 1.0000x reference)
"""Trainium2 Bass kernel for nn_DynamicMoERoutingLayer.

Math: the MoE layer computes, per batch element b,
  os[b] = sum_e softmax_w[b,e] * (conv2d(x[b], conv_w[e]) + conv_b[e]) / sum_e softmax_w[b,e]
Since the conv is linear in the kernel, we combine the E=10 expert kernels into ONE
effective kernel per batch element (W_eff[b] = sum_e w[b,e] conv_w[e]) and run a single
stride-2 3x3 conv per batch element.  This cuts compute 10x and makes the problem
memory-bound (per spec target_regime).

Sharding: data-parallel over batch. 8 cores x 4 batch elements, no collectives.

Conv-as-matmul layout (per core, per PAIR of batch elements):
  x is column-parity-split on the host: stride-2 output taps read single-parity columns.
  SBUF rhs tile xg[128, 8192] fp16 holds, per 32-partition group:
     [ even-cols(b_lo) | odd-cols(b_lo) | even-cols(b_hi) | odd-cols(b_hi) ]
  For each kh and each 8-row output chunk, two BLOCK-DIAGONAL K=128 M=128 matmuls
  accumulate all 3 kw taps for BOTH batch elements of the pair:
     mm_A: taps (kh,0),(kh,1)  -- weight rows [ci(kw0)|ci(kw1)] per 64-row block
     mm_B: tap  (kh,2)         -- even-col view shifted one element; odd rows zero
  Full-array (unmasked) matmuls keep the PE HAM activity monitor engaged so the
  clock gate opens to 2.4 GHz (masked tile_position matmuls do not register).
"""

import numpy as np

B, CIN, H, W = 32, 32, 128, 128
COUT, E, RDIM = 64, 10, 128
OH = OW = 63
NCORES = 8
BPC = B // NCORES          # batch per core = 4
NPAIR = BPC // 2           # pairs per core = 2

# routing-constants pack (fp32, [128, WRT_COLS]) — small, loaded first
_O_W1T = 0            # [128, 128]
_O_W2T = 128          # [128, 64]
_O_B1 = 192           # [128, 1]
_O_B2 = 193           # [64, 1]
_O_EMB = 194          # [10, 64]
_O_CB = 258           # [10, 128]
_O_RVT = 386          # [128, 4]
WRT_COLS = 390

_CACHE = {}


def _build_program(enable_asserts=False):
    import concourse.bacc as bacc
    import concourse.bass as bass
    import concourse.mybir as mybir
    import concourse.tile as tile
    from concourse.masks import make_identity

    f32 = mybir.dt.float32
    f16 = mybir.dt.float16
    FT = mybir.ActivationFunctionType
    ALU = mybir.AluOpType

    nc = bacc.Bacc(
        "TRN2",
        target_bir_lowering=False,
        debug=False,
        enable_asserts=enable_asserts,
        num_devices=NCORES,
    )

    xs_d = nc.dram_tensor("xs", [NPAIR, 128, H * (W // 2)], f16, kind="ExternalInput")
    wrt_d = nc.dram_tensor("wrt", [128, WRT_COLS], f32, kind="ExternalInput")
    # expert conv kernels, A-taps (kw0|kw1) and B-tap (kw2) packed: [128, E, 6, 64]
    cw_d = nc.dram_tensor("cw", [128, E, 6, COUT], f16, kind="ExternalInput")
    out_d = nc.dram_tensor("out", [NPAIR, 128, OH, OW], f32, kind="ExternalOutput")

    with tile.TileContext(nc) as tc:
        with (
            tc.tile_pool(name="consts", bufs=1) as consts,
            tc.tile_pool(name="work", bufs=1) as work,
            tc.tile_pool(name="xin", bufs=2) as xin,
            tc.tile_pool(name="outp", bufs=2) as outp,
            tc.tile_pool(name="rps", bufs=2, space="PSUM") as rps,
            tc.tile_pool(name="cps", bufs=4, space="PSUM") as cps,
        ):
            # ---- constant loads: small routing pack first, expert kernels second ----
            wrt = consts.tile([128, WRT_COLS], f32)
            nc.sync.dma_start(wrt, wrt_d.ap())
            w1t_s = wrt[:, _O_W1T : _O_W1T + 128]
            w2t_s = wrt[:, _O_W2T : _O_W2T + COUT]
            b1_s = wrt[:, _O_B1 : _O_B1 + 1]
            b2_s = wrt[0:COUT, _O_B2 : _O_B2 + 1]
            emb_s = wrt[0:E, _O_EMB : _O_EMB + COUT]
            cb_s = wrt[0:E, _O_CB : _O_CB + 128]
            rvt_s = wrt[:, _O_RVT : _O_RVT + BPC]
            cw_s = consts.tile([128, E, 6, COUT], f16)
            nc.sync.dma_start(cw_s, cw_d.ap())

            ident = consts.tile([COUT, COUT], f32)
            make_identity(nc, ident)
            # half-ones rows for per-64-partition-half broadcasts (K=1 matmuls)
            ones_lo = consts.tile([1, 128], f16)
            nc.vector.memset(ones_lo[:, 0:64], 1.0)
            nc.vector.memset(ones_lo[:, 64:128], 0.0)
            ones_hi = consts.tile([1, 128], f16)
            nc.vector.memset(ones_hi[:, 0:64], 0.0)
            nc.vector.memset(ones_hi[:, 64:128], 1.0)
            ident16 = consts.tile([16, 16], f16)
            nc.vector.tensor_copy(ident16, ident[:16, :16])
            cb16 = consts.tile([E, 128], f16)
            nc.vector.tensor_copy(cb16, cb_s)
            # pre-trigger the Sqrt ACT table load off the critical path (the
            # lower_act pass reloads on every function-group change, so only
            # the first group is worth preloading; Exp then costs one load).
            tdum = consts.tile([1, 2], f32)
            nc.scalar.activation(tdum[:, 0:1], ones_lo[:, 0:1], FT.Sqrt)

            # ---- x loads early, on their own DMA rings ----
            xgs = []
            for pair in range(NPAIR):
                xg = xin.tile([128, H * (W // 2)], f16, tag=f"xg{pair}", name=f"xg{pair}")
                nc.sync.dma_start(xg, xs_d.ap()[pair])
                xgs.append(xg)

            # ---- routing MLP:  r = relu(rv @ W1.T + b1) @ W2.T + b2  (transposed) ----
            ps1 = rps.tile([128, BPC], f32, tag="rp")
            nc.tensor.matmul(ps1, w1t_s, rvt_s, start=True, stop=True)
            h_s = work.tile([128, BPC], f32)
            nc.scalar.activation(h_s, ps1, FT.Relu, bias=b1_s, scale=1.0)

            ps2 = rps.tile([COUT, BPC], f32, tag="rp")
            nc.tensor.matmul(ps2, w2t_s, h_s, start=True, stop=True)
            r_s = work.tile([COUT, BPC], f32)
            nc.scalar.activation(r_s, ps2, FT.Identity, bias=b2_s, scale=1.0)

            # ---- normalize expert embeddings (rows of emb) ----
            esq = work.tile([E, COUT], f32)
            en2 = work.tile([E, 1], f32)
            nc.vector.scalar_tensor_tensor(
                esq, emb_s, 1.0, emb_s, ALU.mult, ALU.mult, accum_out=en2
            )
            ens = work.tile([E, 1], f32)
            nc.scalar.sqrt(ens, en2)
            enr = work.tile([E, 1], f32)
            nc.vector.reciprocal(enr, ens)
            enu = work.tile([E, COUT], f32)
            nc.vector.tensor_scalar_mul(enu, emb_s, enr)
            ps3 = rps.tile([COUT, E], f32, tag="rp")
            nc.tensor.transpose(ps3, enu, ident[:E, :E])
            ent_s = work.tile([COUT, E], f32)
            nc.scalar.copy(ent_s, ps3)

            # ---- 1/|r| per batch element ----
            ps4 = rps.tile([BPC, COUT], f32, tag="rp")
            nc.tensor.transpose(ps4, r_s, ident)
            rT_s = work.tile([BPC, COUT], f32)
            nc.scalar.copy(rT_s, ps4)
            rsq = work.tile([BPC, COUT], f32)
            rn2 = work.tile([BPC, 1], f32)
            nc.vector.scalar_tensor_tensor(
                rsq, rT_s, 1.0, rT_s, ALU.mult, ALU.mult, accum_out=rn2
            )
            rns = work.tile([BPC, 1], f32)
            nc.scalar.sqrt(rns, rn2)
            rni = work.tile([BPC, 1], f32)
            nc.vector.reciprocal(rni, rns)

            # ---- softmax numerator/denominator.
            # ex = exp(cos_sim) (|cos_sim|<=1, no max-subtraction needed);
            # sinv = 1/sum_e ex.  (softmax sums to 1, so the reference's extra
            # division by sum(softmax) is a no-op up to 1e-7.)
            ps5 = rps.tile([BPC, E], f32, tag="rp")
            nc.tensor.matmul(ps5, r_s, ent_s, start=True, stop=True)
            ex = work.tile([BPC, E], f32)
            ssum = work.tile([BPC, 1], f32)
            nc.scalar.activation(ex, ps5, FT.Exp, scale=rni, accum_out=ssum)
            sinv = work.tile([BPC, 1], f32)
            nc.vector.reciprocal(sinv, ssum)

            # ---- broadcast ex rows and sinv across partitions, on-chip.
            # psrow[1, :] = [ex0 | ex2 | ex1 | ex3 | s0 s1 s2 s3] via transposes,
            # then half-ones K=1 matmuls give per-partition-half values:
            #   ps8[p, pair*10+e] = ex[2*pair + (p>=64), e]
            #   ps8b[p, pair]     = sinv[2*pair + (p>=64)]
            ps6 = rps.tile([E, BPC], f32, tag="rp")
            nc.tensor.transpose(ps6, ex, ident[:BPC, :BPC])
            exT_s = work.tile([E, BPC], f16)
            nc.scalar.copy(exT_s, ps6)

            psrow = rps.tile([1, BPC * E + BPC], f16, tag="rp")
            colmap = {0: 0, 2: 1, 1: 2, 3: 3}
            for b in range(BPC):
                j = colmap[b]
                nc.tensor.transpose(
                    psrow[0:1, E * j : E * (j + 1)], exT_s[:, b : b + 1], ident16[:E, :E]
                )
            sinv16 = work.tile([BPC, 1], f16)
            nc.vector.tensor_copy(sinv16, sinv)
            nc.tensor.transpose(
                psrow[0:1, BPC * E : BPC * E + BPC], sinv16, ident16[:BPC, :BPC]
            )
            srow = work.tile([1, BPC * E + BPC], f16)
            nc.scalar.copy(srow, psrow)

            ps8 = rps.tile([128, NPAIR * E], f32, tag="rp")
            nc.tensor.matmul(ps8, ones_lo, srow[0:1, 0 : NPAIR * E], start=True, stop=False)
            nc.tensor.matmul(
                ps8, ones_hi, srow[0:1, NPAIR * E : 2 * NPAIR * E], start=False, stop=True
            )
            sv = srow[:, BPC * E : BPC * E + BPC].rearrange("p (a two) -> p a two", two=2)
            ps8b = rps.tile([128, NPAIR], f32, tag="rp")
            nc.tensor.matmul(ps8b, ones_lo, sv[:, :, 0], start=True, stop=False)
            nc.tensor.matmul(ps8b, ones_hi, sv[:, :, 1], start=False, stop=True)

            wbc2 = work.tile([128, NPAIR, E], f32)
            nc.vector.tensor_copy(wbc2, ps8.rearrange("p (b e) -> p b e", e=E))
            sinv_bc = work.tile([128, NPAIR], f32)
            nc.vector.tensor_copy(sinv_bc, ps8b)

            # ---- effective conv bias:  beff[m, b] = sinv[b] * sum_e conv_b[e, m%64]*ex[b, e]
            ps7 = rps.tile([128, BPC], f32, tag="rp")
            nc.tensor.matmul(ps7, cb16, exT_s, start=True, stop=True)
            beff_s = work.tile([128, BPC], f32)
            nc.scalar.copy(beff_s, ps7)
            beff2 = work.tile([128, NPAIR], f32)
            for pair in range(NPAIR):
                nc.scalar.mul(
                    beff2[0:64, pair : pair + 1],
                    beff_s[0:64, 2 * pair : 2 * pair + 1],
                    mul=sinv_bc[0:64, pair : pair + 1],
                )
                nc.scalar.mul(
                    beff2[64:128, pair : pair + 1],
                    beff_s[64:128, 2 * pair + 1 : 2 * pair + 2],
                    mul=sinv_bc[64:128, pair : pair + 1],
                )

            # ---- PE warm-up burst overlapping the combine, so the HAM clock
            # gate is at 8/8 when the conv starts.
            cwf = cw_s.rearrange("p e t c -> p (e t c)")
            wsrc = work.tile([128, NPAIR * E], f16)
            nc.scalar.copy(wsrc, ps8)
            wps = cps.tile([NPAIR * E, 504], f32, tag="warm", bufs=1)
            for _ in range(23):
                nc.tensor.matmul(wps, wsrc, cwf[:, 0:504], start=True, stop=True)

            # ---- combine expert kernels (un-normalized ex weights):
            #   acc[p, pair, t, co] = sum_e wbc2[p,pair,e] * cw[p,e,t,co]   (t: 3 A + 3 B)
            acc = work.tile([128, NPAIR, 6, COUT], f16)
            # block-diagonal fp16 stationary tiles, normalized by sinv during cast
            bd16a = work.tile([128, NPAIR, 3, 128], f16)
            bd16b = work.tile([128, NPAIR, 3, 128], f16)
            nc.vector.memset(bd16a, 0.0)
            nc.vector.memset(bd16b, 0.0)
            prev_cast = None
            for pair in range(NPAIR):
                for e in range(E):
                    sc = wbc2[:, pair, e : e + 1]
                    if e == 0:
                        i0 = nc.vector.tensor_scalar_mul(acc[:, pair], cw_s[:, e], sc)
                        if prev_cast is not None:
                            # keep the DVE on pair0's chain until its bd16 tiles are
                            # cast, so pair0's conv matmuls start ~4us earlier
                            bass._add_dep_helper(
                                i0.ins, prev_cast.ins, sync=True,
                                reason="pair1 combine after pair0 cast",
                            )
                    else:
                        nc.vector.scalar_tensor_tensor(
                            acc[:, pair], cw_s[:, e], sc, acc[:, pair], ALU.mult, ALU.add
                        )
                sc_lo = sinv_bc[0:64, pair : pair + 1]
                sc_hi = sinv_bc[64:128, pair : pair + 1]
                nc.vector.tensor_scalar_mul(
                    bd16a[0:64, pair, :, 0:64], acc[0:64, pair, 0:3, :], sc_lo
                )
                nc.vector.tensor_scalar_mul(
                    bd16a[64:128, pair, :, 64:128], acc[64:128, pair, 0:3, :], sc_hi
                )
                nc.vector.tensor_scalar_mul(
                    bd16b[0:32, pair, :, 0:64], acc[0:32, pair, 3:6, :], sc_lo[0:32]
                )
                prev_cast = nc.vector.tensor_scalar_mul(
                    bd16b[64:96, pair, :, 64:128], acc[64:96, pair, 3:6, :],
                    sinv_bc[64:96, pair : pair + 1],
                )

            # ---- the conv ----
            KHSEL = ((0, 0), (0, 1), (1, 0))  # kh -> (dh, s):  h = 2*oh + kh = 2*(oh+dh) + s
            for pair in range(NPAIR):
                xg = xgs[pair]
                xv = xg.rearrange("p (ho s wo) -> p ho s wo", s=2, wo=W // 2)
                stage = outp.tile([128, OH, OW], f32, tag="stage")
                for c in range(8):
                    oh0 = 8 * c
                    nr = min(8, OH - oh0)
                    ps = cps.tile([128, 8, OW], f32, tag="cps")
                    for kh in range(3):
                        dh, s = KHSEL[kh]
                        rhs_a = xv[:, oh0 + dh : oh0 + dh + nr, s, 0:OW]
                        rhs_b = xv[:, oh0 + dh : oh0 + dh + nr, s, 1 : OW + 1]
                        nc.tensor.matmul(
                            ps[:, 0:nr, :], bd16a[:, pair, kh, :], rhs_a,
                            start=(kh == 0), stop=False,
                        )
                        nc.tensor.matmul(
                            ps[:, 0:nr, :], bd16b[:, pair, kh, :], rhs_b,
                            start=False, stop=(kh == 2),
                        )
                    nc.scalar.activation(
                        stage[:, oh0 : oh0 + nr, :], ps[:, 0:nr, :], FT.Identity,
                        bias=beff2[:, pair : pair + 1], scale=1.0,
                    )
                    if c == 3:
                        nc.sync.dma_start(out_d.ap()[pair, :, 0:32, :], stage[:, 0:32, :])
                    elif c == 5:
                        nc.sync.dma_start(out_d.ap()[pair, :, 32:48, :], stage[:, 32:48, :])
                    elif c == 6:
                        nc.sync.dma_start(out_d.ap()[pair, :, 48:56, :], stage[:, 48:56, :])
                    elif c == 7:
                        nc.sync.dma_start(out_d.ap()[pair, :, 56:OH, :], stage[:, 56:OH, :])

    nc.compile()
    return nc


def _get_program():
    if "nc" not in _CACHE:
        _CACHE["nc"] = _build_program()
    return _CACHE["nc"]


def _prep_shards(x, routing_vector, W1, b1, W2, b2, conv_w, conv_b, emb):
    """Host-side layout transforms. Returns list of 8 per-core input dicts."""
    x = np.ascontiguousarray(np.asarray(x, dtype=np.float32))
    rv = np.asarray(routing_vector, dtype=np.float32)
    W1 = np.asarray(W1, dtype=np.float32)
    b1 = np.asarray(b1, dtype=np.float32)
    W2 = np.asarray(W2, dtype=np.float32)
    b2 = np.asarray(b2, dtype=np.float32)
    conv_w = np.asarray(conv_w, dtype=np.float32)
    conv_b = np.asarray(conv_b, dtype=np.float32)
    emb = np.asarray(emb, dtype=np.float32)

    # x: [B, CI, H, W] -> parity split [B, 2, CI, H*(W/2)] fp16
    xv = x.reshape(B, CIN, H, W // 2, 2).transpose(0, 4, 1, 2, 3)
    xs_all = np.ascontiguousarray(xv, dtype=np.float16).reshape(B, 2, CIN, H * (W // 2))

    # conv_w [E, CO, CI, KH, KW] -> t [KW, CI, E, KH, CO]
    t = np.ascontiguousarray(conv_w.transpose(4, 2, 0, 3, 1))
    z = np.zeros_like(t[2])
    # [128(p), E, kh, CO] for A-taps (kw of group parity) and B-tap (kw2 / zeros)
    cwa = np.concatenate([t[0], t[1], t[0], t[1]], axis=0)
    cwb = np.concatenate([t[2], z, t[2], z], axis=0)
    cw = np.ascontiguousarray(
        np.concatenate([cwa, cwb], axis=2), dtype=np.float16  # [128, E, 6, CO]
    )

    wrt_base = np.zeros((128, WRT_COLS), dtype=np.float32)
    wrt_base[:, _O_W1T : _O_W1T + 128] = W1.T
    wrt_base[:, _O_W2T : _O_W2T + COUT] = W2.T
    wrt_base[:, _O_B1] = b1
    wrt_base[0:COUT, _O_B2] = b2
    wrt_base[0:E, _O_EMB : _O_EMB + COUT] = emb
    wrt_base[0:E, _O_CB : _O_CB + 128] = np.tile(conv_b, (1, 2))

    in_maps = []
    for i in range(NCORES):
        sl = slice(4 * i, 4 * i + 4)
        # partition order within a pair is (b2, parity, ci):
        # xs_all[sl] is [4b, par, ci, :] -> [pair, b2, par, ci, :] -> [pair, 128, :]
        xs = np.ascontiguousarray(xs_all[sl]).reshape(NPAIR, 128, H * (W // 2))
        wrt = wrt_base.copy()
        wrt[:, _O_RVT : _O_RVT + BPC] = rv[sl].T
        in_maps.append({"xs": xs, "wrt": wrt, "cw": cw})
    return in_maps


def kernel(x, routing_vector, task=None, W1=None, b1=None, W2=None, b2=None,
           conv_w=None, conv_b=None, emb=None, _trace=False):
    from concourse.bass_utils import run_bass_kernel_spmd

    nc = _get_program()
    in_maps = _prep_shards(x, routing_vector, W1, b1, W2, b2, conv_w, conv_b, emb)
    res = run_bass_kernel_spmd(
        nc, in_maps, core_ids=list(range(NCORES)), trace=_trace,
        trace_cores=[0] if _trace else None,
    )
    _CACHE["last_results"] = res
    out = np.empty((B, COUT, OH, OW), dtype=np.float32)
    for i in range(NCORES):
        o = res.results[i]["out"].reshape(BPC, COUT, OH, OW)
        out[4 * i : 4 * i + 4] = o
    return out


# revision 24
# speedup vs baseline: 1.0181x; 1.0181x over previous
"""Trainium2 Bass kernel for nn_DynamicMoERoutingLayer.

Math: the MoE layer computes, per batch element b,
  os[b] = sum_e softmax_w[b,e] * (conv2d(x[b], conv_w[e]) + conv_b[e]) / sum_e softmax_w[b,e]
Since the conv is linear in the kernel, we combine the E=10 expert kernels into ONE
effective kernel per batch element (W_eff[b] = sum_e w[b,e] conv_w[e]) and run a single
stride-2 3x3 conv per batch element.  This cuts compute 10x and makes the problem
memory-bound (per spec target_regime).

Sharding: data-parallel over batch. 8 cores x 4 batch elements, no collectives.

Conv-as-matmul layout (per core, per PAIR of batch elements):
  x is column-parity-split on the host: stride-2 output taps read single-parity columns.
  SBUF rhs tile xg[128, 8192] fp16 holds, per 32-partition group:
     [ even-cols(b_lo) | odd-cols(b_lo) | even-cols(b_hi) | odd-cols(b_hi) ]
  For each kh and each 8-row output chunk, two BLOCK-DIAGONAL K=128 M=128 matmuls
  accumulate all 3 kw taps for BOTH batch elements of the pair:
     mm_A: taps (kh,0),(kh,1)  -- weight rows [ci(kw0)|ci(kw1)] per 64-row block
     mm_B: tap  (kh,2)         -- even-col view shifted one element; odd rows zero
  Full-array (unmasked) matmuls keep the PE HAM activity monitor engaged so the
  clock gate opens to 2.4 GHz (masked tile_position matmuls do not register);
  a warm-up matmul burst overlapping the expert-combine keeps it open.

  Orchestration notes (measured on trn2u via axon):
   - all input DMAs share one HWDGE ring in priority order (routing consts,
     expert kernels, x pair0, x pair1) so each gets full HBM bandwidth in turn;
   - the expert-combine (10-term MAC chain on the vector engine) runs per pair
     with pair0 first (explicit dep), so pair0's conv overlaps pair1's combine;
   - softmax weights are broadcast across partitions on-chip: row-transposes to
     partition 0, then two accumulating K=1 matmuls with half-ones vectors;
   - activation-table thrash is minimized (vector-engine squares, one Sqrt
     preload dummy, fp16 broadcast matmuls), leaving one Exp table load.
"""

import numpy as np

B, CIN, H, W = 32, 32, 128, 128
COUT, E, RDIM = 64, 10, 128
OH = OW = 63
NCORES = 8
BPC = B // NCORES          # batch per core = 4
NPAIR = BPC // 2           # pairs per core = 2

# routing-constants pack (fp32, [128, WRT_COLS]) — small, loaded first
_O_W1T = 0            # [128, 128]
_O_W2T = 128          # [128, 64]
_O_B1 = 192           # [128, 1]
_O_B2 = 193           # [64, 1]
_O_EMB = 194          # [10, 64]
_O_CB = 258           # [10, 128]
_O_RVT = 386          # [128, 4]
WRT_COLS = 390

_CACHE = {}


def _build_program(enable_asserts=False):
    import concourse.bacc as bacc
    import concourse.bass as bass
    import concourse.mybir as mybir
    import concourse.tile as tile
    from concourse.masks import make_identity

    f32 = mybir.dt.float32
    f16 = mybir.dt.float16
    FT = mybir.ActivationFunctionType
    ALU = mybir.AluOpType

    nc = bacc.Bacc(
        "TRN2",
        target_bir_lowering=False,
        debug=False,
        enable_asserts=enable_asserts,
        num_devices=NCORES,
    )

    xs_d = nc.dram_tensor("xs", [NPAIR, 128, H * (W // 2)], f16, kind="ExternalInput")
    wrt_d = nc.dram_tensor("wrt", [128, WRT_COLS], f32, kind="ExternalInput")
    # expert conv kernels, A-taps (kw0|kw1) and B-tap (kw2) packed: [128, E, 6, 64]
    cw_d = nc.dram_tensor("cw", [128, E, 6, COUT], f16, kind="ExternalInput")
    out_d = nc.dram_tensor("out", [NPAIR, 128, OH, OW], f32, kind="ExternalOutput")

    with tile.TileContext(nc) as tc:
        with (
            tc.tile_pool(name="consts", bufs=1) as consts,
            tc.tile_pool(name="work", bufs=1) as work,
            tc.tile_pool(name="xin", bufs=2) as xin,
            tc.tile_pool(name="outp", bufs=2) as outp,
            tc.tile_pool(name="rps", bufs=2, space="PSUM") as rps,
            tc.tile_pool(name="cps", bufs=4, space="PSUM") as cps,
        ):
            # ---- constant loads: small routing pack first, expert kernels second ----
            wrt = consts.tile([128, WRT_COLS], f32)
            nc.sync.dma_start(wrt, wrt_d.ap())
            w1t_s = wrt[:, _O_W1T : _O_W1T + 128]
            w2t_s = wrt[:, _O_W2T : _O_W2T + COUT]
            b1_s = wrt[:, _O_B1 : _O_B1 + 1]
            b2_s = wrt[0:COUT, _O_B2 : _O_B2 + 1]
            emb_s = wrt[0:E, _O_EMB : _O_EMB + COUT]
            cb_s = wrt[0:E, _O_CB : _O_CB + 128]
            rvt_s = wrt[:, _O_RVT : _O_RVT + BPC]
            cw_s = consts.tile([128, E, 6, COUT], f16)
            nc.sync.dma_start(cw_s, cw_d.ap())

            ident = consts.tile([COUT, COUT], f32)
            make_identity(nc, ident)
            # half-ones rows for per-64-partition-half broadcasts (K=1 matmuls)
            ones_lo = consts.tile([1, 128], f16)
            nc.vector.memset(ones_lo[:, 0:64], 1.0)
            nc.vector.memset(ones_lo[:, 64:128], 0.0)
            ones_hi = consts.tile([1, 128], f16)
            nc.vector.memset(ones_hi[:, 0:64], 0.0)
            nc.vector.memset(ones_hi[:, 64:128], 1.0)
            ident16 = consts.tile([16, 16], f16)
            nc.vector.tensor_copy(ident16, ident[:16, :16])
            cb16 = consts.tile([E, 128], f16)
            nc.vector.tensor_copy(cb16, cb_s)
            # pre-trigger the Sqrt ACT table load off the critical path (the
            # lower_act pass reloads on every function-group change, so only
            # the first group is worth preloading; Exp then costs one load).
            tdum = consts.tile([1, 2], f32)
            nc.scalar.activation(tdum[:, 0:1], ones_lo[:, 0:1], FT.Sqrt)

            # ---- x loads early, on their own DMA rings ----
            xgs = []
            for pair in range(NPAIR):
                xg = xin.tile([128, H * (W // 2)], f16, tag=f"xg{pair}", name=f"xg{pair}")
                nc.sync.dma_start(xg, xs_d.ap()[pair])
                xgs.append(xg)

            # ---- routing MLP:  r = relu(rv @ W1.T + b1) @ W2.T + b2  (transposed) ----
            ps1 = rps.tile([128, BPC], f32, tag="rp")
            nc.tensor.matmul(ps1, w1t_s, rvt_s, start=True, stop=True)
            h_s = work.tile([128, BPC], f32)
            nc.scalar.activation(h_s, ps1, FT.Relu, bias=b1_s, scale=1.0)

            ps2 = rps.tile([COUT, BPC], f32, tag="rp")
            nc.tensor.matmul(ps2, w2t_s, h_s, start=True, stop=True)
            r_s = work.tile([COUT, BPC], f32)
            nc.scalar.activation(r_s, ps2, FT.Identity, bias=b2_s, scale=1.0)

            # ---- normalize expert embeddings (rows of emb) ----
            esq = work.tile([E, COUT], f32)
            en2 = work.tile([E, 1], f32)
            nc.vector.scalar_tensor_tensor(
                esq, emb_s, 1.0, emb_s, ALU.mult, ALU.mult, accum_out=en2
            )
            ens = work.tile([E, 1], f32)
            nc.scalar.sqrt(ens, en2)
            enr = work.tile([E, 1], f32)
            nc.vector.reciprocal(enr, ens)
            enu = work.tile([E, COUT], f32)
            nc.vector.tensor_scalar_mul(enu, emb_s, enr)
            ps3 = rps.tile([COUT, E], f32, tag="rp")
            nc.tensor.transpose(ps3, enu, ident[:E, :E])
            ent_s = work.tile([COUT, E], f32)
            nc.scalar.copy(ent_s, ps3)

            # ---- 1/|r| per batch element ----
            ps4 = rps.tile([BPC, COUT], f32, tag="rp")
            nc.tensor.transpose(ps4, r_s, ident)
            rT_s = work.tile([BPC, COUT], f32)
            nc.scalar.copy(rT_s, ps4)
            rsq = work.tile([BPC, COUT], f32)
            rn2 = work.tile([BPC, 1], f32)
            nc.vector.scalar_tensor_tensor(
                rsq, rT_s, 1.0, rT_s, ALU.mult, ALU.mult, accum_out=rn2
            )
            rns = work.tile([BPC, 1], f32)
            nc.scalar.sqrt(rns, rn2)
            rni = work.tile([BPC, 1], f32)
            nc.vector.reciprocal(rni, rns)

            # ---- softmax numerator/denominator.
            # ex = exp(cos_sim) (|cos_sim|<=1, no max-subtraction needed);
            # sinv = 1/sum_e ex.  (softmax sums to 1, so the reference's extra
            # division by sum(softmax) is a no-op up to 1e-7.)
            ps5 = rps.tile([BPC, E], f32, tag="rp")
            nc.tensor.matmul(ps5, r_s, ent_s, start=True, stop=True)
            ex = work.tile([BPC, E], f32)
            ssum = work.tile([BPC, 1], f32)
            nc.scalar.activation(ex, ps5, FT.Exp, scale=rni, accum_out=ssum)
            sinv = work.tile([BPC, 1], f32)
            nc.vector.reciprocal(sinv, ssum)

            # ---- broadcast ex rows and sinv across partitions, on-chip.
            # psrow[1, :] = [ex0 | ex2 | ex1 | ex3 | s0 s1 s2 s3] via transposes,
            # then half-ones K=1 matmuls give per-partition-half values:
            #   ps8[p, pair*10+e] = ex[2*pair + (p>=64), e]
            #   ps8b[p, pair]     = sinv[2*pair + (p>=64)]
            ps6 = rps.tile([E, BPC], f32, tag="rp")
            nc.tensor.transpose(ps6, ex, ident[:BPC, :BPC])
            exT_s = work.tile([E, BPC], f16)
            nc.scalar.copy(exT_s, ps6)

            psrow = rps.tile([1, BPC * E + BPC], f16, tag="rp")
            colmap = {0: 0, 2: 1, 1: 2, 3: 3}
            for b in range(BPC):
                j = colmap[b]
                nc.tensor.transpose(
                    psrow[0:1, E * j : E * (j + 1)], exT_s[:, b : b + 1], ident16[:E, :E]
                )
            sinv16 = work.tile([BPC, 1], f16)
            nc.vector.tensor_copy(sinv16, sinv)
            nc.tensor.transpose(
                psrow[0:1, BPC * E : BPC * E + BPC], sinv16, ident16[:BPC, :BPC]
            )
            srow = work.tile([1, BPC * E + BPC], f16)
            nc.scalar.copy(srow, psrow)

            ps8 = rps.tile([128, NPAIR * E], f32, tag="rp")
            nc.tensor.matmul(ps8, ones_lo, srow[0:1, 0 : NPAIR * E], start=True, stop=False)
            nc.tensor.matmul(
                ps8, ones_hi, srow[0:1, NPAIR * E : 2 * NPAIR * E], start=False, stop=True
            )
            sv = srow[:, BPC * E : BPC * E + BPC].rearrange("p (a two) -> p a two", two=2)
            ps8b = rps.tile([128, NPAIR], f32, tag="rp")
            nc.tensor.matmul(ps8b, ones_lo, sv[:, :, 0], start=True, stop=False)
            nc.tensor.matmul(ps8b, ones_hi, sv[:, :, 1], start=False, stop=True)

            wbc2 = work.tile([128, NPAIR, E], f32)
            nc.vector.tensor_copy(wbc2, ps8.rearrange("p (b e) -> p b e", e=E))
            sinv_bc = work.tile([128, NPAIR], f32)
            nc.vector.tensor_copy(sinv_bc, ps8b)

            # ---- effective conv bias:  beff[m, b] = sinv[b] * sum_e conv_b[e, m%64]*ex[b, e]
            ps7 = rps.tile([128, BPC], f32, tag="rp")
            nc.tensor.matmul(ps7, cb16, exT_s, start=True, stop=True)
            beff_s = work.tile([128, BPC], f32)
            nc.scalar.copy(beff_s, ps7)
            beff2 = work.tile([128, NPAIR], f32)
            for pair in range(NPAIR):
                nc.scalar.mul(
                    beff2[0:64, pair : pair + 1],
                    beff_s[0:64, 2 * pair : 2 * pair + 1],
                    mul=sinv_bc[0:64, pair : pair + 1],
                )
                nc.scalar.mul(
                    beff2[64:128, pair : pair + 1],
                    beff_s[64:128, 2 * pair + 1 : 2 * pair + 2],
                    mul=sinv_bc[64:128, pair : pair + 1],
                )

            # ---- PE warm-up burst overlapping the combine, so the HAM clock
            # gate is at 8/8 when the conv starts.
            cwf = cw_s.rearrange("p e t c -> p (e t c)")
            wsrc = work.tile([128, NPAIR * E], f16)
            nc.scalar.copy(wsrc, ps8)
            wps = cps.tile([NPAIR * E, 504], f32, tag="warm", bufs=1)
            for _ in range(23):
                nc.tensor.matmul(wps, wsrc, cwf[:, 0:504], start=True, stop=True)

            # ---- combine expert kernels (un-normalized ex weights):
            #   acc[p, pair, t, co] = sum_e wbc2[p,pair,e] * cw[p,e,t,co]   (t: 3 A + 3 B)
            acc = work.tile([128, NPAIR, 6, COUT], f16)
            # block-diagonal fp16 stationary tiles, normalized by sinv during cast
            bd16a = work.tile([128, NPAIR, 3, 128], f16)
            bd16b = work.tile([128, NPAIR, 3, 128], f16)
            nc.vector.memset(bd16a, 0.0)
            nc.vector.memset(bd16b, 0.0)
            prev_cast = None
            for pair in range(NPAIR):
                for e in range(E):
                    sc = wbc2[:, pair, e : e + 1]
                    if e == 0:
                        i0 = nc.vector.tensor_scalar_mul(acc[:, pair], cw_s[:, e], sc)
                        if prev_cast is not None:
                            # keep the DVE on pair0's chain until its bd16 tiles are
                            # cast, so pair0's conv matmuls start ~4us earlier
                            bass._add_dep_helper(
                                i0.ins, prev_cast.ins, sync=True,
                                reason="pair1 combine after pair0 cast",
                            )
                    else:
                        nc.vector.scalar_tensor_tensor(
                            acc[:, pair], cw_s[:, e], sc, acc[:, pair], ALU.mult, ALU.add
                        )
                sc_lo = sinv_bc[0:64, pair : pair + 1]
                sc_hi = sinv_bc[64:128, pair : pair + 1]
                nc.vector.tensor_scalar_mul(
                    bd16a[0:64, pair, :, 0:64], acc[0:64, pair, 0:3, :], sc_lo
                )
                nc.vector.tensor_scalar_mul(
                    bd16a[64:128, pair, :, 64:128], acc[64:128, pair, 0:3, :], sc_hi
                )
                nc.vector.tensor_scalar_mul(
                    bd16b[0:32, pair, :, 0:64], acc[0:32, pair, 3:6, :], sc_lo[0:32]
                )
                prev_cast = nc.vector.tensor_scalar_mul(
                    bd16b[64:96, pair, :, 64:128], acc[64:96, pair, 3:6, :],
                    sinv_bc[64:96, pair : pair + 1],
                )

            # ---- the conv ----
            KHSEL = ((0, 0), (0, 1), (1, 0))  # kh -> (dh, s):  h = 2*oh + kh = 2*(oh+dh) + s
            for pair in range(NPAIR):
                xg = xgs[pair]
                xv = xg.rearrange("p (ho s wo) -> p ho s wo", s=2, wo=W // 2)
                stage = outp.tile([128, OH, OW], f32, tag="stage")
                for c in range(8):
                    oh0 = 8 * c
                    nr = min(8, OH - oh0)
                    ps = cps.tile([128, 8, OW], f32, tag="cps")
                    for kh in range(3):
                        dh, s = KHSEL[kh]
                        rhs_a = xv[:, oh0 + dh : oh0 + dh + nr, s, 0:OW]
                        rhs_b = xv[:, oh0 + dh : oh0 + dh + nr, s, 1 : OW + 1]
                        nc.tensor.matmul(
                            ps[:, 0:nr, :], bd16a[:, pair, kh, :], rhs_a,
                            start=(kh == 0), stop=False,
                        )
                        nc.tensor.matmul(
                            ps[:, 0:nr, :], bd16b[:, pair, kh, :], rhs_b,
                            start=False, stop=(kh == 2),
                        )
                    nc.scalar.activation(
                        stage[:, oh0 : oh0 + nr, :], ps[:, 0:nr, :], FT.Identity,
                        bias=beff2[:, pair : pair + 1], scale=1.0,
                    )
                    if c == 3:
                        nc.sync.dma_start(out_d.ap()[pair, :, 0:32, :], stage[:, 0:32, :])
                    elif c == 5:
                        nc.sync.dma_start(out_d.ap()[pair, :, 32:48, :], stage[:, 32:48, :])
                    elif c == 6:
                        nc.sync.dma_start(out_d.ap()[pair, :, 48:56, :], stage[:, 48:56, :])
                    elif c == 7:
                        nc.sync.dma_start(out_d.ap()[pair, :, 56:OH, :], stage[:, 56:OH, :])

    nc.compile()
    return nc


def _get_program():
    if "nc" not in _CACHE:
        _CACHE["nc"] = _build_program()
    return _CACHE["nc"]


def _prep_shards(x, routing_vector, W1, b1, W2, b2, conv_w, conv_b, emb):
    """Host-side layout transforms. Returns list of 8 per-core input dicts."""
    x = np.ascontiguousarray(np.asarray(x, dtype=np.float32))
    rv = np.asarray(routing_vector, dtype=np.float32)
    W1 = np.asarray(W1, dtype=np.float32)
    b1 = np.asarray(b1, dtype=np.float32)
    W2 = np.asarray(W2, dtype=np.float32)
    b2 = np.asarray(b2, dtype=np.float32)
    conv_w = np.asarray(conv_w, dtype=np.float32)
    conv_b = np.asarray(conv_b, dtype=np.float32)
    emb = np.asarray(emb, dtype=np.float32)

    # x: [B, CI, H, W] -> parity split [B, 2, CI, H*(W/2)] fp16
    xv = x.reshape(B, CIN, H, W // 2, 2).transpose(0, 4, 1, 2, 3)
    xs_all = np.ascontiguousarray(xv, dtype=np.float16).reshape(B, 2, CIN, H * (W // 2))

    # conv_w [E, CO, CI, KH, KW] -> t [KW, CI, E, KH, CO]
    t = np.ascontiguousarray(conv_w.transpose(4, 2, 0, 3, 1))
    z = np.zeros_like(t[2])
    # [128(p), E, kh, CO] for A-taps (kw of group parity) and B-tap (kw2 / zeros)
    cwa = np.concatenate([t[0], t[1], t[0], t[1]], axis=0)
    cwb = np.concatenate([t[2], z, t[2], z], axis=0)
    cw = np.ascontiguousarray(
        np.concatenate([cwa, cwb], axis=2), dtype=np.float16  # [128, E, 6, CO]
    )

    wrt_base = np.zeros((128, WRT_COLS), dtype=np.float32)
    wrt_base[:, _O_W1T : _O_W1T + 128] = W1.T
    wrt_base[:, _O_W2T : _O_W2T + COUT] = W2.T
    wrt_base[:, _O_B1] = b1
    wrt_base[0:COUT, _O_B2] = b2
    wrt_base[0:E, _O_EMB : _O_EMB + COUT] = emb
    wrt_base[0:E, _O_CB : _O_CB + 128] = np.tile(conv_b, (1, 2))

    in_maps = []
    for i in range(NCORES):
        sl = slice(4 * i, 4 * i + 4)
        # partition order within a pair is (b2, parity, ci):
        # xs_all[sl] is [4b, par, ci, :] -> [pair, b2, par, ci, :] -> [pair, 128, :]
        xs = np.ascontiguousarray(xs_all[sl]).reshape(NPAIR, 128, H * (W // 2))
        wrt = wrt_base.copy()
        wrt[:, _O_RVT : _O_RVT + BPC] = rv[sl].T
        in_maps.append({"xs": xs, "wrt": wrt, "cw": cw})
    return in_maps


def kernel(x, routing_vector, task=None, W1=None, b1=None, W2=None, b2=None,
           conv_w=None, conv_b=None, emb=None, _trace=False):
    from concourse.bass_utils import run_bass_kernel_spmd

    nc = _get_program()
    in_maps = _prep_shards(x, routing_vector, W1, b1, W2, b2, conv_w, conv_b, emb)
    res = run_bass_kernel_spmd(
        nc, in_maps, core_ids=list(range(NCORES)), trace=_trace,
        trace_cores=[0] if _trace else None,
    )
    _CACHE["last_results"] = res
    out = np.empty((B, COUT, OH, OW), dtype=np.float32)
    for i in range(NCORES):
        o = res.results[i]["out"].reshape(BPC, COUT, OH, OW)
        out[4 * i : 4 * i + 4] = o
    return out
